# revision 18
# baseline (speedup 1.0000x reference)
"""Trainium2 Bass kernel for single-head attention.

Reference computation (per batch b):
    q = x @ Wq; k = x @ Wk; v = x @ Wv          # x: [S, D], W: [D, D]
    out = softmax(q @ k.T / sqrt(D)) @ v

Shapes: B=4, S=2048, D=1024, f32.

Sharding over 8 NeuronCores: core c -> (batch b = c//2, seq half h = c%2).
Each core:
  - computes q^T, k^T (layout [e, s]) and v ([s, e]) for its own S/2 rows
  - AllGathers k^T (float32r) and v (bf16) within the pair {2b, 2b+1},
    chunked into <=2MB collectives (mesh algorithm + early start)
  - computes scores for its 1024 queries vs all 2048 keys, softmax,
    attn @ v, writes its [1024, 1024] output shard.

dtype strategy (validated empirically):
  - all matmuls in float32r (TensorE full rate at free-dim>=256,
    ~13-bit mantissa; measured end-to-end rel err ~9e-3 vs the f32
    reference, under the 2e-2 gate; plain f32 matmul is 4x slower)
  - attn weights / gathered v in bf16 (error enters output linearly).

Scheduling notes:
  - Weight loads ride the sync (HWDGE) queue as f32->f32r bitcasts so
    nothing queues behind the gpsimd collective triggers.
  - Attention phase is software-pipelined: TensorE stream is
    scores(i), transposes(i-1), attn@v(i-1); the DVE stream runs the
    transpose copies before tile i's rowmax so PE never starves.
"""

import numpy as np

import concourse.bass as bass
import concourse.mybir as mybir
import concourse.tile as tile
from concourse import bacc
from concourse.bass_utils import run_bass_kernel_spmd

P = 128          # partitions
D = 1024         # model dim (= E)
S_OWN = 1024     # sequence rows per core
S_FULL = 2048
B, NCORES = 4, 8
DT = D // P      # 8 d-tiles
ST = S_OWN // P  # 8 s-tiles
NT = S_FULL // P  # 16 key tiles
F32 = mybir.dt.float32
F32R = mybir.dt.float32r
BF16 = mybir.dt.bfloat16
REPLICA_GROUPS = [[0, 1], [2, 3], [4, 5], [6, 7]]
KT_CHUNKS = 2    # k^T AllGather split (2MB each -> mesh algorithm)
V_CHUNKS = 2     # v AllGather split (1MB each)


def build_kernel():
    nc = bacc.Bacc("TRN2", target_bir_lowering=False, num_devices=NCORES)

    x_d = nc.dram_tensor("x", [S_OWN, D], F32, kind="ExternalInput")
    wq_d = nc.dram_tensor("Wq", [D, D], F32, kind="ExternalInput")
    wk_d = nc.dram_tensor("Wk", [D, D], F32, kind="ExternalInput")
    wv_d = nc.dram_tensor("Wv", [D, D], F32, kind="ExternalInput")
    out_d = nc.dram_tensor("out", [S_OWN, D], F32, kind="ExternalOutput")

    # collective bounce buffers (internal DRAM), chunked along e/s
    ec = D // KT_CHUNKS   # e-rows per kT chunk
    sc = S_OWN // V_CHUNKS
    kt_send = [nc.dram_tensor(f"kt_send{i}", [ec, S_OWN], F32R)
               for i in range(KT_CHUNKS)]
    kt_allc = [nc.dram_tensor(f"kt_all{i}", [2, ec, S_OWN], F32R)
               for i in range(KT_CHUNKS)]
    v_send = [nc.dram_tensor(f"v_send{i}", [sc, D], BF16)
              for i in range(V_CHUNKS)]
    v_allc = [nc.dram_tensor(f"v_all{i}", [2, sc, D], BF16)
              for i in range(V_CHUNKS)]

    bar_send = nc.dram_tensor("bar_send", [1, 128], F32)
    bar_out = nc.dram_tensor("bar_out", [2, 128], F32)

    ident_np = np.eye(P, dtype=np.float32)
    ident_d = nc.inline_tensor(ident_np, name="ident")

    with tile.TileContext(nc) as tc:
        _emit(nc, tc, x_d, wq_d, wk_d, wv_d, out_d,
              kt_send, kt_allc, v_send, v_allc, ident_d, bar_send, bar_out)
    nc.compile()
    return nc


def _emit(nc, tc, x_d, wq_d, wk_d, wv_d, out_d,
          kt_send, kt_allc, v_send, v_allc, ident_d, bar_send, bar_out):
    with tc.tile_pool(name="sb", bufs=1) as sb:
        ident = sb.tile([P, P], F32, name="ident")
        nc.sync.dma_start(ident[:], ident_d.ap())
        identb = sb.tile([P, P], BF16, name="identb")
        nc.gpsimd.dma_start(identb[:], ident_d.ap())  # cast f32->bf16

        # SBUF tag plan (KB/partition, 207.9 usable):
        #  idents ~1 | xa: x_nat->attn 12 | big4k: xT(8)->kTg(16) 80
        #  | wkwq: wk,wv->wq 64 | qT 32 | kts 4 | vs 2 | attnT 8 | ost 4
        # pair barrier: tiny AllGather so later collectives see no skew
        nc.gpsimd.dma_start(bar_send.ap(), ident_d.ap()[0:1, :])
        nc.gpsimd.collective_compute(
            "AllGather", mybir.AluOpType.bypass,
            replica_groups=REPLICA_GROUPS,
            ins=[bar_send.ap().opt()],
            outs=[bar_out.ap().opt()],
        )

        xT = [sb.tile([P, S_OWN], F32R, name=f"xT{d}", tag="big4k", bufs=16)
              for d in range(DT)]
        wq_sb = [sb.tile([P, D], F32R, name=f"wq{d}", tag="big4k", bufs=16)
                 for d in range(DT)]
        wk_sb = [sb.tile([P, D], F32R, name=f"wk{d}", tag="wkv", bufs=16)
                 for d in range(DT)]
        wv_sb = [sb.tile([P, D], F32R, name=f"wv{d}", tag="wkv", bufs=16)
                 for d in range(DT)]
        qT = [sb.tile([P, S_OWN], F32R, name=f"qT{e}", tag="qT", bufs=8)
              for e in range(DT)]

        with tc.tile_pool(name="ps1", bufs=1, space="PSUM") as ps1:
            # ---- load x (split across both HWDGE queues), transpose ----
            x_nats = []
            for s in range(ST):
                x_nat = sb.tile([P, D], F32, name=f"x_nat{s}", tag="xa",
                                bufs=3)
                eng = nc.sync if s % 2 == 0 else nc.scalar
                eng.dma_start(x_nat[:], x_d.ap()[s * P:(s + 1) * P, :])
                x_nats.append(x_nat)

            # weight loads on the scalar HWDGE queue, after the x tiles
            # (f32 -> f32r bitcast: same bytes; PE truncates on read).
            # wk first (needed earliest), then wv, then wq.
            for d in range(DT):
                nc.scalar.dma_start(
                    wk_sb[d][:], wk_d.ap()[d * P:(d + 1) * P, :].bitcast(F32R))
            for d in range(DT):
                nc.scalar.dma_start(
                    wv_sb[d][:], wv_d.ap()[d * P:(d + 1) * P, :].bitcast(F32R))
            for d in range(DT):
                nc.scalar.dma_start(
                    wq_sb[d][:], wq_d.ap()[d * P:(d + 1) * P, :].bitcast(F32R))

            for s in range(ST):
                x_nat = x_nats[s]
                for d in range(DT):
                    pt = ps1.tile([P, P], F32, name=f"pt{s}_{d}", tag="pt",
                                  bufs=2)
                    nc.tensor.transpose(pt[:], x_nat[:, d * P:(d + 1) * P],
                                        ident[:])
                    nc.vector.tensor_copy(xT[d][:, s * P:(s + 1) * P], pt[:])

            # ---- k^T projection (f32r) -> DRAM -> chunked AllGather ----
            epc = DT // KT_CHUNKS  # e-tiles per chunk
            for ch in range(KT_CHUNKS):
                for ei in range(epc):
                    e = ch * epc + ei
                    pk = ps1.tile([P, S_OWN], F32, name=f"pk{e}", tag="proj",
                                  bufs=3)
                    for d in range(DT):
                        for c in range(2):
                            nc.tensor.matmul(
                                pk[:, c * 512:(c + 1) * 512],
                                wk_sb[d][:, e * P:(e + 1) * P],
                                xT[d][:, c * 512:(c + 1) * 512],
                                start=(d == 0), stop=(d == DT - 1))
                    kt_stage = sb.tile([P, S_OWN], F32R, name=f"kts{e}",
                                       tag="stage", bufs=2)
                    nc.vector.tensor_copy(kt_stage[:], pk[:])
                    nc.sync.dma_start(kt_send[ch].ap()[ei * P:(ei + 1) * P, :],
                                      kt_stage[:])
                nc.gpsimd.collective_compute(
                    "AllGather", mybir.AluOpType.bypass,
                    replica_groups=REPLICA_GROUPS,
                    ins=[kt_send[ch].ap().opt()],
                    outs=[kt_allc[ch].ap().opt()],
                )

            # ---- v projection (f32r in, bf16 out) -> chunked AllGather ----
            spc = ST // V_CHUNKS
            for ch in range(V_CHUNKS):
                for si in range(spc):
                    s = ch * spc + si
                    pv = ps1.tile([P, D], F32, name=f"pv{s}", tag="proj",
                                  bufs=3)
                    for d in range(DT):
                        for c in range(2):
                            nc.tensor.matmul(
                                pv[:, c * 512:(c + 1) * 512],
                                xT[d][:, s * P:(s + 1) * P],
                                wv_sb[d][:, c * 512:(c + 1) * 512],
                                start=(d == 0), stop=(d == DT - 1))
                    v_stage = sb.tile([P, D], BF16, name=f"vs{s}", tag="stage", bufs=2)
                    nc.vector.tensor_copy(v_stage[:], pv[:])
                    nc.sync.dma_start(v_send[ch].ap()[si * P:(si + 1) * P, :],
                                      v_stage[:])
                nc.gpsimd.collective_compute(
                    "AllGather", mybir.AluOpType.bypass,
                    replica_groups=REPLICA_GROUPS,
                    ins=[v_send[ch].ap().opt()],
                    outs=[v_allc[ch].ap().opt()],
                )

            # ---- q^T projection (f32r), kept in SBUF ----
            for e in range(DT):
                pq = ps1.tile([P, S_OWN], F32, name=f"pq{e}", tag="proj",
                              bufs=3)
                for d in range(DT):
                    for c in range(2):
                        nc.tensor.matmul(
                            pq[:, c * 512:(c + 1) * 512],
                            wq_sb[d][:, e * P:(e + 1) * P],
                            xT[d][:, c * 512:(c + 1) * 512],
                            start=(d == 0), stop=(d == DT - 1))
                nc.vector.tensor_copy(qT[e][:], pq[:])

        # ---- bring gathered k^T / v into SBUF (chunk-wise) ----
        kT_sb = [[sb.tile([P, S_OWN], F32R, name=f"kTg{r}_{e}", tag="big4k",
                          bufs=16) for e in range(DT)] for r in range(2)]
        v_sb = [[sb.tile([P, D], BF16, name=f"vg{r}_{s}", tag="wkv", bufs=16)
                 for s in range(ST)] for r in range(2)]
        epc = DT // KT_CHUNKS
        spc = ST // V_CHUNKS
        for ch in range(KT_CHUNKS):
            for ei in range(epc):
                e = ch * epc + ei
                for r in range(2):
                    eng = nc.sync if r == 0 else nc.scalar
                    eng.dma_start(
                        kT_sb[r][e][:],
                        kt_allc[ch].ap()[r, ei * P:(ei + 1) * P, :])
        for ch in range(V_CHUNKS):
            for r in range(2):
                for si in range(spc):
                    s = ch * spc + si
                    nc.scalar.dma_start(
                        v_sb[r][s][:],
                        v_allc[ch].ap()[r, si * P:(si + 1) * P, :])

        # ---- attention, software-pipelined over 128-query tiles ----
        with tc.tile_pool(name="ps2", bufs=1, space="PSUM") as ps2:
            state = {}

            def emit_scores(sq):
                S_ps = ps2.tile([P, S_FULL], F32, name=f"S{sq}", tag="S",
                                bufs=1)
                for e in range(DT):
                    for r in range(2):
                        for c in range(2):
                            col = r * S_OWN + c * 512
                            nc.tensor.matmul(
                                S_ps[:, col:col + 512],
                                qT[e][:, sq * P:(sq + 1) * P],
                                kT_sb[r][e][:, c * 512:(c + 1) * 512],
                                start=(e == 0), stop=(e == DT - 1))
                state[sq] = S_ps

            def emit_softmax(sq):
                S_ps = state.pop(sq)
                m = sb.tile([P, 1], F32, name=f"m{sq}", tag="m", bufs=2)
                nc.vector.reduce_max(m[:], S_ps[:], axis=mybir.AxisListType.X)
                negm = sb.tile([P, 1], F32, name=f"negm{sq}", tag="negm",
                               bufs=2)
                nc.scalar.mul(negm[:], m[:], -1.0 / 32.0)
                attn = sb.tile([P, S_FULL], BF16, name=f"attn{sq}", tag="xa",
                               bufs=3)
                lsum = sb.tile([P, 1], F32, name=f"lsum{sq}", tag="lsum",
                               bufs=2)
                nc.scalar.activation(
                    attn[:], S_ps[:], mybir.ActivationFunctionType.Exp,
                    bias=negm[:, 0:1], scale=1.0 / 32.0, accum_out=lsum[:])
                rl = sb.tile([P, 1], F32, name=f"rl{sq}", tag="rl", bufs=2)
                nc.vector.reciprocal(rl[:], lsum[:])
                state[(sq, "sm")] = (attn, rl)

            def emit_transp(sq):
                attn, rl = state[(sq, "sm")]
                attnT = sb.tile([P, S_FULL], BF16, name=f"attnT{sq}",
                                tag="attnT", bufs=2)
                for t in range(NT):
                    pat = ps2.tile([P, P], BF16, name=f"pat{sq}_{t}",
                                   tag="pat", bufs=2)
                    nc.tensor.transpose(
                        pat[:], attn[:, t * P:(t + 1) * P], identb[:])
                    nc.vector.tensor_copy(attnT[:, t * P:(t + 1) * P], pat[:])
                state[(sq, "T")] = attnT

            def emit_avmm(sq):
                attn, rl = state.pop((sq, "sm"))
                attnT = state.pop((sq, "T"))
                O_ps = ps2.tile([P, D], F32, name=f"O{sq}", tag="O", bufs=1)
                for t in range(NT):
                    r, s = divmod(t, ST)
                    for c in range(2):
                        nc.tensor.matmul(
                            O_ps[:, c * 512:(c + 1) * 512],
                            attnT[:, t * P:(t + 1) * P],
                            v_sb[r][s][:, c * 512:(c + 1) * 512],
                            start=(t == 0), stop=(t == NT - 1))
                o_stage = sb.tile([P, D], F32, name=f"ost{sq}", tag="stage", bufs=2)
                nc.vector.tensor_scalar_mul(o_stage[:], O_ps[:], rl[:, 0:1])
                nc.sync.dma_start(out_d.ap()[sq * P:(sq + 1) * P, :],
                                  o_stage[:])

            # PE stream: T(i-1) | scores(i) | AV(i-1); the pat->attnT
            # copies run on DVE while scores(i) occupies the PE.
            emit_scores(0)
            emit_softmax(0)
            for sq in range(1, ST):
                emit_transp(sq - 1)
                emit_scores(sq)
                emit_avmm(sq - 1)
                emit_softmax(sq)
            emit_transp(ST - 1)
            emit_avmm(ST - 1)


_NC_CACHE = {}


def _get_nc():
    if "nc" not in _NC_CACHE:
        _NC_CACHE["nc"] = build_kernel()
    return _NC_CACHE["nc"]


def kernel(x, Wq, Wk, Wv, **_ignored):
    x = np.ascontiguousarray(np.asarray(x, dtype=np.float32))
    Wq = np.ascontiguousarray(np.asarray(Wq, dtype=np.float32))
    Wk = np.ascontiguousarray(np.asarray(Wk, dtype=np.float32))
    Wv = np.ascontiguousarray(np.asarray(Wv, dtype=np.float32))
    nc = _get_nc()
    in_maps = []
    for c in range(NCORES):
        b, h = divmod(c, 2)
        in_maps.append({
            "x": x[b, h * S_OWN:(h + 1) * S_OWN, :],
            "Wq": Wq, "Wk": Wk, "Wv": Wv,
        })
    res = run_bass_kernel_spmd(nc, in_maps, core_ids=list(range(NCORES)))
    out = np.empty((B, S_FULL, D), dtype=np.float32)
    for c in range(NCORES):
        b, h = divmod(c, 2)
        out[b, h * S_OWN:(h + 1) * S_OWN, :] = res.results[c]["out"]
    return out


# revision 19
# speedup vs baseline: 1.0455x; 1.0455x over previous
"""Trainium2 Bass kernel for single-head attention.

Reference computation (per batch b):
    q = x @ Wq; k = x @ Wk; v = x @ Wv          # x: [S, D], W: [D, D]
    out = softmax(q @ k.T / sqrt(D)) @ v

Shapes: B=4, S=2048, D=1024, f32.

Sharding over 8 NeuronCores: core c -> (batch b = c//2, seq half h = c%2).
Each core:
  - computes q^T, k^T (layout [e, s]) and v ([s, e]) for its own S/2 rows
  - AllGathers k^T (float32r) and v (bf16) within the pair {2b, 2b+1},
    chunked into <=2MB collectives (mesh algorithm + early start)
  - computes scores for its 1024 queries vs all 2048 keys, softmax,
    attn @ v, writes its [1024, 1024] output shard.

dtype strategy (validated empirically):
  - all matmuls in float32r (TensorE full rate at free-dim>=256,
    ~13-bit mantissa; measured end-to-end rel err ~9e-3 vs the f32
    reference, under the 2e-2 gate; plain f32 matmul is 4x slower)
  - attn weights / gathered v in bf16 (error enters output linearly).

Scheduling notes:
  - Weight loads ride the sync (HWDGE) queue as f32->f32r bitcasts so
    nothing queues behind the gpsimd collective triggers.
  - Attention phase is software-pipelined: TensorE stream is
    scores(i), transposes(i-1), attn@v(i-1); the DVE stream runs the
    transpose copies before tile i's rowmax so PE never starves.
"""

import numpy as np

import concourse.bass as bass
import concourse.mybir as mybir
import concourse.tile as tile
from concourse import bacc
from concourse.bass_utils import run_bass_kernel_spmd

P = 128          # partitions
D = 1024         # model dim (= E)
S_OWN = 1024     # sequence rows per core
S_FULL = 2048
B, NCORES = 4, 8
DT = D // P      # 8 d-tiles
ST = S_OWN // P  # 8 s-tiles
NT = S_FULL // P  # 16 key tiles
F32 = mybir.dt.float32
F32R = mybir.dt.float32r
BF16 = mybir.dt.bfloat16
REPLICA_GROUPS = [[0, 1], [2, 3], [4, 5], [6, 7]]
KT_CHUNKS = 2    # k^T AllGather split (2MB each -> mesh algorithm)
V_CHUNKS = 2     # v AllGather split (1MB each)


def build_kernel():
    nc = bacc.Bacc("TRN2", target_bir_lowering=False, num_devices=NCORES)

    x_d = nc.dram_tensor("x", [S_OWN, D], F32, kind="ExternalInput")
    wq_d = nc.dram_tensor("Wq", [D, D], F32, kind="ExternalInput")
    wk_d = nc.dram_tensor("Wk", [D, D], F32, kind="ExternalInput")
    wv_d = nc.dram_tensor("Wv", [D, D], F32, kind="ExternalInput")
    out_d = nc.dram_tensor("out", [S_OWN, D], F32, kind="ExternalOutput")

    # collective bounce buffers (internal DRAM), chunked along e/s
    ec = D // KT_CHUNKS   # e-rows per kT chunk
    sc = S_OWN // V_CHUNKS
    kt_send = [nc.dram_tensor(f"kt_send{i}", [ec, S_OWN], F32R)
               for i in range(KT_CHUNKS)]
    kt_allc = [nc.dram_tensor(f"kt_all{i}", [2, ec, S_OWN], F32R)
               for i in range(KT_CHUNKS)]
    v_send = [nc.dram_tensor(f"v_send{i}", [sc, D], BF16)
              for i in range(V_CHUNKS)]
    v_allc = [nc.dram_tensor(f"v_all{i}", [2, sc, D], BF16)
              for i in range(V_CHUNKS)]

    bar_send = nc.dram_tensor("bar_send", [1, 128], F32)
    bar_out = nc.dram_tensor("bar_out", [2, 128], F32)

    ident_np = np.eye(P, dtype=np.float32)
    ident_d = nc.inline_tensor(ident_np, name="ident")

    with tile.TileContext(nc) as tc:
        _emit(nc, tc, x_d, wq_d, wk_d, wv_d, out_d,
              kt_send, kt_allc, v_send, v_allc, ident_d, bar_send, bar_out)
    nc.compile()
    return nc


def _emit(nc, tc, x_d, wq_d, wk_d, wv_d, out_d,
          kt_send, kt_allc, v_send, v_allc, ident_d, bar_send, bar_out):
    with tc.tile_pool(name="sb", bufs=1) as sb:
        ident = sb.tile([P, P], F32, name="ident")
        nc.sync.dma_start(ident[:], ident_d.ap())
        identb = sb.tile([P, P], BF16, name="identb")
        nc.gpsimd.dma_start(identb[:], ident_d.ap())  # cast f32->bf16

        # pair barrier: tiny AllGather so later collectives see no skew
        nc.gpsimd.dma_start(bar_send.ap(), ident_d.ap()[0:1, :])
        nc.gpsimd.collective_compute(
            "AllGather", mybir.AluOpType.bypass,
            replica_groups=REPLICA_GROUPS,
            ins=[bar_send.ap().opt()],
            outs=[bar_out.ap().opt()],
        )

        # SBUF tag plan (KB/partition, 207.9 usable). Generational reuse:
        #   wqwk: wq(8), wk(8) -> kTg(16)   [wq dies at q-end, wk at kT-end,
        #                                    exactly when gathered k chunks land]
        #   xTvg: xT(8) -> vg r0(8)         [xT dies at v-end]
        #   wvvg: wv(8) -> vg r1(8)
        #   xa: x_nat -> attn | qT | stage: kts,vs,ost | attnT
        wq_sb = [sb.tile([P, D], F32R, name=f"wq{d}", tag="wqwk", bufs=16)
                 for d in range(DT)]
        wk_sb = [sb.tile([P, D], F32R, name=f"wk{d}", tag="wqwk", bufs=16)
                 for d in range(DT)]
        xT = [sb.tile([P, S_OWN], F32R, name=f"xT{d}", tag="xTvg", bufs=8)
              for d in range(DT)]
        wv_sb = [sb.tile([P, D], F32R, name=f"wv{d}", tag="wvvg", bufs=8)
                 for d in range(DT)]
        qT = [sb.tile([P, S_OWN], F32R, name=f"qT{e}", tag="qT", bufs=8)
              for e in range(DT)]

        # weight loads on the scalar HWDGE queue (f32 -> f32r bitcast:
        # same bytes; PE truncates the mantissa on read); none has a
        # slot-reuse wait, so the queue never blocks.
        for d in range(DT):
            nc.scalar.dma_start(wq_sb[d][:],
                                wq_d.ap()[d * P:(d + 1) * P, :].bitcast(F32R))
        for d in range(DT):
            nc.scalar.dma_start(wk_sb[d][:],
                                wk_d.ap()[d * P:(d + 1) * P, :].bitcast(F32R))
        for d in range(DT):
            nc.scalar.dma_start(wv_sb[d][:],
                                wv_d.ap()[d * P:(d + 1) * P, :].bitcast(F32R))

        with tc.tile_pool(name="ps1", bufs=1, space="PSUM") as ps1:
            # ---- load x (sync queue), transpose to xT (f32r) ----
            for s in range(ST):
                x_nat = sb.tile([P, D], F32, name=f"x_nat{s}", tag="xa",
                                bufs=3)
                nc.sync.dma_start(x_nat[:], x_d.ap()[s * P:(s + 1) * P, :])
                for d in range(DT):
                    pt = ps1.tile([P, P], F32, name=f"pt{s}_{d}", tag="pt",
                                  bufs=4)
                    nc.tensor.transpose(pt[:], x_nat[:, d * P:(d + 1) * P],
                                        ident[:])
                    nc.vector.tensor_copy(xT[d][:, s * P:(s + 1) * P], pt[:])

            # ---- q^T projection first (wq slots free earliest) ----
            for e in range(DT):
                pq = ps1.tile([P, S_OWN], F32, name=f"pq{e}", tag="proj",
                              bufs=2)
                for d in range(DT):
                    for c in range(2):
                        nc.tensor.matmul(
                            pq[:, c * 512:(c + 1) * 512],
                            wq_sb[d][:, e * P:(e + 1) * P],
                            xT[d][:, c * 512:(c + 1) * 512],
                            start=(d == 0), stop=(d == DT - 1))
                nc.vector.tensor_copy(qT[e][:], pq[:])

            # ---- k^T projection -> DRAM -> chunked AllGather ----
            epc = DT // KT_CHUNKS
            for ch in range(KT_CHUNKS):
                for ei in range(epc):
                    e = ch * epc + ei
                    pk = ps1.tile([P, S_OWN], F32, name=f"pk{e}", tag="proj",
                                  bufs=2)
                    for d in range(DT):
                        for c in range(2):
                            nc.tensor.matmul(
                                pk[:, c * 512:(c + 1) * 512],
                                wk_sb[d][:, e * P:(e + 1) * P],
                                xT[d][:, c * 512:(c + 1) * 512],
                                start=(d == 0), stop=(d == DT - 1))
                    kt_stage = sb.tile([P, S_OWN], F32R, name=f"kts{e}",
                                       tag="stage", bufs=2)
                    nc.vector.tensor_copy(kt_stage[:], pk[:])
                    nc.sync.dma_start(kt_send[ch].ap()[ei * P:(ei + 1) * P, :],
                                      kt_stage[:])
                nc.gpsimd.collective_compute(
                    "AllGather", mybir.AluOpType.bypass,
                    replica_groups=REPLICA_GROUPS,
                    ins=[kt_send[ch].ap().opt()],
                    outs=[kt_allc[ch].ap().opt()],
                )

            # ---- v projection -> DRAM -> AllGather ----
            spc = ST // V_CHUNKS
            for ch in range(V_CHUNKS):
                for si in range(spc):
                    s = ch * spc + si
                    pv = ps1.tile([P, D], F32, name=f"pv{s}", tag="proj",
                                  bufs=2)
                    for d in range(DT):
                        for c in range(2):
                            nc.tensor.matmul(
                                pv[:, c * 512:(c + 1) * 512],
                                xT[d][:, s * P:(s + 1) * P],
                                wv_sb[d][:, c * 512:(c + 1) * 512],
                                start=(d == 0), stop=(d == DT - 1))
                    v_stage = sb.tile([P, D], BF16, name=f"vs{s}", tag="stage", bufs=2)
                    nc.vector.tensor_copy(v_stage[:], pv[:])
                    nc.sync.dma_start(v_send[ch].ap()[si * P:(si + 1) * P, :],
                                      v_stage[:])
                nc.gpsimd.collective_compute(
                    "AllGather", mybir.AluOpType.bypass,
                    replica_groups=REPLICA_GROUPS,
                    ins=[v_send[ch].ap().opt()],
                    outs=[v_allc[ch].ap().opt()],
                )

        # ---- bring gathered k^T / v into SBUF ----
        # kTg: into wq slots (ch0) / wk slots (ch1); r0 on sync, r1 on scalar
        kT_sb_flat = {}
        epc = DT // KT_CHUNKS
        for ch in range(KT_CHUNKS):
            for ei in range(epc):
                e = ch * epc + ei
                for r in range(2):
                    t = sb.tile([P, S_OWN], F32R, name=f"kTg{r}_{e}",
                                tag="wqwk", bufs=16)
                    kT_sb_flat[(r, e)] = t
                    eng = nc.sync if r == 0 else nc.scalar
                    eng.dma_start(
                        t[:], kt_allc[ch].ap()[r, ei * P:(ei + 1) * P, :])
        kT_sb = [[kT_sb_flat[(r, e)] for e in range(DT)] for r in range(2)]

        # vg: r0 into xT slots, r1 into wv slots; all on sync
        v_sb = []
        for r in range(2):
            row = []
            for s in range(ST):
                t = sb.tile([P, D], BF16, name=f"vg{r}_{s}",
                            tag=("xTvg" if r == 0 else "wvvg"),
                            bufs=(8 if r == 0 else 8))
                row.append(t)
            v_sb.append(row)
        spc = ST // V_CHUNKS
        for ch in range(V_CHUNKS):
            for si in range(spc):
                s = ch * spc + si
                for r in range(2):
                    nc.sync.dma_start(
                        v_sb[r][s][:],
                        v_allc[ch].ap()[r, si * P:(si + 1) * P, :])

        # ---- attention, software-pipelined over 128-query tiles ----
        with tc.tile_pool(name="ps2", bufs=1, space="PSUM") as ps2:
            state = {}

            def emit_scores(sq):
                S_ps = ps2.tile([P, S_FULL], F32, name=f"S{sq}", tag="S",
                                bufs=1)
                for e in range(DT):
                    for r in range(2):
                        for c in range(2):
                            col = r * S_OWN + c * 512
                            nc.tensor.matmul(
                                S_ps[:, col:col + 512],
                                qT[e][:, sq * P:(sq + 1) * P],
                                kT_sb[r][e][:, c * 512:(c + 1) * 512],
                                start=(e == 0), stop=(e == DT - 1))
                state[sq] = S_ps

            def emit_softmax(sq):
                S_ps = state.pop(sq)
                m = sb.tile([P, 1], F32, name=f"m{sq}", tag="m", bufs=2)
                nc.vector.reduce_max(m[:], S_ps[:], axis=mybir.AxisListType.X)
                negm = sb.tile([P, 1], F32, name=f"negm{sq}", tag="negm",
                               bufs=2)
                nc.scalar.mul(negm[:], m[:], -1.0 / 32.0)
                attn = sb.tile([P, S_FULL], BF16, name=f"attn{sq}", tag="xa",
                               bufs=3)
                lsum = sb.tile([P, 1], F32, name=f"lsum{sq}", tag="lsum",
                               bufs=2)
                nc.scalar.activation(
                    attn[:], S_ps[:], mybir.ActivationFunctionType.Exp,
                    bias=negm[:, 0:1], scale=1.0 / 32.0, accum_out=lsum[:])
                rl = sb.tile([P, 1], F32, name=f"rl{sq}", tag="rl", bufs=2)
                nc.vector.reciprocal(rl[:], lsum[:])
                state[(sq, "sm")] = (attn, rl)

            def emit_transp(sq):
                attn, rl = state[(sq, "sm")]
                attnT = sb.tile([P, S_FULL], BF16, name=f"attnT{sq}",
                                tag="attnT", bufs=2)
                for t in range(NT):
                    pat = ps2.tile([P, P], BF16, name=f"pat{sq}_{t}",
                                   tag="pat", bufs=2)
                    nc.tensor.transpose(
                        pat[:], attn[:, t * P:(t + 1) * P], identb[:])
                    nc.vector.tensor_copy(attnT[:, t * P:(t + 1) * P], pat[:])
                state[(sq, "T")] = attnT

            def emit_avmm(sq):
                attn, rl = state.pop((sq, "sm"))
                attnT = state.pop((sq, "T"))
                O_ps = ps2.tile([P, D], F32, name=f"O{sq}", tag="O", bufs=1)
                for t in range(NT):
                    r, s = divmod(t, ST)
                    for c in range(2):
                        nc.tensor.matmul(
                            O_ps[:, c * 512:(c + 1) * 512],
                            attnT[:, t * P:(t + 1) * P],
                            v_sb[r][s][:, c * 512:(c + 1) * 512],
                            start=(t == 0), stop=(t == NT - 1))
                o_stage = sb.tile([P, D], F32, name=f"ost{sq}", tag="stage", bufs=2)
                nc.vector.tensor_scalar_mul(o_stage[:], O_ps[:], rl[:, 0:1])
                nc.sync.dma_start(out_d.ap()[sq * P:(sq + 1) * P, :],
                                  o_stage[:])

            # PE stream: T(i-1) | scores(i) | AV(i-1); the pat->attnT
            # copies run on DVE while scores(i) occupies the PE.
            emit_scores(0)
            emit_softmax(0)
            for sq in range(1, ST):
                emit_transp(sq - 1)
                emit_scores(sq)
                emit_avmm(sq - 1)
                emit_softmax(sq)
            emit_transp(ST - 1)
            emit_avmm(ST - 1)


_NC_CACHE = {}


def _get_nc():
    if "nc" not in _NC_CACHE:
        _NC_CACHE["nc"] = build_kernel()
    return _NC_CACHE["nc"]


def kernel(x, Wq, Wk, Wv, **_ignored):
    x = np.ascontiguousarray(np.asarray(x, dtype=np.float32))
    Wq = np.ascontiguousarray(np.asarray(Wq, dtype=np.float32))
    Wk = np.ascontiguousarray(np.asarray(Wk, dtype=np.float32))
    Wv = np.ascontiguousarray(np.asarray(Wv, dtype=np.float32))
    nc = _get_nc()
    in_maps = []
    for c in range(NCORES):
        b, h = divmod(c, 2)
        in_maps.append({
            "x": x[b, h * S_OWN:(h + 1) * S_OWN, :],
            "Wq": Wq, "Wk": Wk, "Wv": Wv,
        })
    res = run_bass_kernel_spmd(nc, in_maps, core_ids=list(range(NCORES)))
    out = np.empty((B, S_FULL, D), dtype=np.float32)
    for c in range(NCORES):
        b, h = divmod(c, 2)
        out[b, h * S_OWN:(h + 1) * S_OWN, :] = res.results[c]["out"]
    return out


# revision 20
# speedup vs baseline: 1.0669x; 1.0204x over previous
"""Trainium2 Bass kernel for single-head attention.

Reference computation (per batch b):
    q = x @ Wq; k = x @ Wk; v = x @ Wv          # x: [S, D], W: [D, D]
    out = softmax(q @ k.T / sqrt(D)) @ v

Shapes: B=4, S=2048, D=1024, f32.

Sharding over 8 NeuronCores: core c -> (batch b = c//2, seq half h = c%2).
Each core:
  - computes q^T, k^T (layout [e, s]) and v ([s, e]) for its own S/2 rows
  - AllGathers k^T (float32r) and v (bf16) within the pair {2b, 2b+1},
    chunked into <=2MB collectives (mesh algorithm + early start)
  - computes scores for its 1024 queries vs all 2048 keys, softmax,
    attn @ v, writes its [1024, 1024] output shard.

dtype strategy (validated empirically):
  - all matmuls in float32r (TensorE full rate at free-dim>=256,
    ~13-bit mantissa; measured end-to-end rel err ~9e-3 vs the f32
    reference, under the 2e-2 gate; plain f32 matmul is 4x slower)
  - attn weights / gathered v in bf16 (error enters output linearly).

Scheduling notes:
  - Weight loads ride the sync (HWDGE) queue as f32->f32r bitcasts so
    nothing queues behind the gpsimd collective triggers.
  - Attention phase is software-pipelined: TensorE stream is
    scores(i), transposes(i-1), attn@v(i-1); the DVE stream runs the
    transpose copies before tile i's rowmax so PE never starves.
"""

import numpy as np

import concourse.bass as bass
import concourse.mybir as mybir
import concourse.tile as tile
from concourse import bacc
from concourse.bass_utils import run_bass_kernel_spmd

P = 128          # partitions
D = 1024         # model dim (= E)
S_OWN = 1024     # sequence rows per core
S_FULL = 2048
B, NCORES = 4, 8
DT = D // P      # 8 d-tiles
ST = S_OWN // P  # 8 s-tiles
NT = S_FULL // P  # 16 key tiles
F32 = mybir.dt.float32
F32R = mybir.dt.float32r
BF16 = mybir.dt.bfloat16
REPLICA_GROUPS = [[0, 1], [2, 3], [4, 5], [6, 7]]
KT_CHUNKS = 2    # k^T AllGather split (2MB each -> mesh algorithm)
V_CHUNKS = 2     # v AllGather split (1MB each)


def build_kernel():
    nc = bacc.Bacc("TRN2", target_bir_lowering=False, num_devices=NCORES)

    x_d = nc.dram_tensor("x", [S_OWN, D], F32, kind="ExternalInput")
    wq_d = nc.dram_tensor("Wq", [D, D], F32, kind="ExternalInput")
    wk_d = nc.dram_tensor("Wk", [D, D], F32, kind="ExternalInput")
    wv_d = nc.dram_tensor("Wv", [D, D], F32, kind="ExternalInput")
    out_d = nc.dram_tensor("out", [S_OWN, D], F32, kind="ExternalOutput")

    # collective bounce buffers (internal DRAM), chunked along e/s
    ec = D // KT_CHUNKS   # e-rows per kT chunk
    sc = S_OWN // V_CHUNKS
    kt_send = [nc.dram_tensor(f"kt_send{i}", [ec, S_OWN], F32R)
               for i in range(KT_CHUNKS)]
    kt_allc = [nc.dram_tensor(f"kt_all{i}", [2, ec, S_OWN], F32R)
               for i in range(KT_CHUNKS)]
    v_send = [nc.dram_tensor(f"v_send{i}", [sc, D], BF16)
              for i in range(V_CHUNKS)]
    v_allc = [nc.dram_tensor(f"v_all{i}", [2, sc, D], BF16)
              for i in range(V_CHUNKS)]

    bar_send = nc.dram_tensor("bar_send", [1, 128], F32)
    bar_out = nc.dram_tensor("bar_out", [2, 128], F32)

    ident_np = np.eye(P, dtype=np.float32)
    ident_d = nc.inline_tensor(ident_np, name="ident")

    with tile.TileContext(nc) as tc:
        _emit(nc, tc, x_d, wq_d, wk_d, wv_d, out_d,
              kt_send, kt_allc, v_send, v_allc, ident_d, bar_send, bar_out)
    nc.compile()
    return nc


def _emit(nc, tc, x_d, wq_d, wk_d, wv_d, out_d,
          kt_send, kt_allc, v_send, v_allc, ident_d, bar_send, bar_out):
    with tc.tile_pool(name="sb", bufs=1) as sb:
        ident = sb.tile([P, P], F32, name="ident")
        nc.sync.dma_start(ident[:], ident_d.ap())
        identb = sb.tile([P, P], BF16, name="identb")
        nc.gpsimd.dma_start(identb[:], ident_d.ap())  # cast f32->bf16

        # pair barrier: tiny AllGather so later collectives see no skew
        nc.gpsimd.dma_start(bar_send.ap(), ident_d.ap()[0:1, :])
        nc.gpsimd.collective_compute(
            "AllGather", mybir.AluOpType.bypass,
            replica_groups=REPLICA_GROUPS,
            ins=[bar_send.ap().opt()],
            outs=[bar_out.ap().opt()],
        )

        # SBUF tag plan (KB/partition, 207.9 usable). Generational reuse:
        #   wqwk: wq(8), wk(8) -> kTg(16)   [wq dies at q-end, wk at kT-end,
        #                                    exactly when gathered k chunks land]
        #   xTvg: xT(8) -> vg r0(8)         [xT dies at v-end]
        #   wvvg: wv(8) -> vg r1(8)
        #   xa: x_nat -> attn | qT | stage: kts,vs,ost | attnT
        wk_sb = [sb.tile([P, D], F32R, name=f"wk{d}", tag="wqwk", bufs=16)
                 for d in range(DT)]
        wq_sb = [sb.tile([P, D], F32R, name=f"wq{d}", tag="wqwk", bufs=16)
                 for d in range(DT)]
        xT = [sb.tile([P, S_OWN], F32R, name=f"xT{d}", tag="xTvg", bufs=8)
              for d in range(DT)]
        wv_sb = [sb.tile([P, D], F32R, name=f"wv{d}", tag="wvvg", bufs=8)
                 for d in range(DT)]
        qT = [sb.tile([P, S_OWN], F32R, name=f"qT{e}", tag="qT", bufs=8)
              for e in range(DT)]

        # weight loads on the scalar HWDGE queue (f32 -> f32r bitcast:
        # same bytes; PE truncates the mantissa on read); none has a
        # slot-reuse wait, so the queue never blocks.
        for d in range(DT):
            nc.scalar.dma_start(wk_sb[d][:],
                                wk_d.ap()[d * P:(d + 1) * P, :].bitcast(F32R))
        for d in range(DT):
            nc.scalar.dma_start(wq_sb[d][:],
                                wq_d.ap()[d * P:(d + 1) * P, :].bitcast(F32R))
        for d in range(DT):
            nc.scalar.dma_start(wv_sb[d][:],
                                wv_d.ap()[d * P:(d + 1) * P, :].bitcast(F32R))

        with tc.tile_pool(name="ps1", bufs=1, space="PSUM") as ps1:
            # ---- load x (sync queue), transpose to xT (f32r) ----
            for s in range(ST):
                x_nat = sb.tile([P, D], F32, name=f"x_nat{s}", tag="xa",
                                bufs=3)
                nc.sync.dma_start(x_nat[:], x_d.ap()[s * P:(s + 1) * P, :])
                for d in range(DT):
                    pt = ps1.tile([P, P], F32, name=f"pt{s}_{d}", tag="pt",
                                  bufs=4)
                    nc.tensor.transpose(pt[:], x_nat[:, d * P:(d + 1) * P],
                                        ident[:])
                    nc.vector.tensor_copy(xT[d][:, s * P:(s + 1) * P], pt[:])

            # ---- k^T projection -> DRAM -> chunked AllGather (early,
            # while the DMA fabric is otherwise quiet) ----
            epc = DT // KT_CHUNKS
            for ch in range(KT_CHUNKS):
                for ei in range(epc):
                    e = ch * epc + ei
                    pk = ps1.tile([P, S_OWN], F32, name=f"pk{e}", tag="proj",
                                  bufs=2)
                    for d in range(DT):
                        for c in range(2):
                            nc.tensor.matmul(
                                pk[:, c * 512:(c + 1) * 512],
                                wk_sb[d][:, e * P:(e + 1) * P],
                                xT[d][:, c * 512:(c + 1) * 512],
                                start=(d == 0), stop=(d == DT - 1))
                    kt_stage = sb.tile([P, S_OWN], F32R, name=f"kts{e}",
                                       tag="stage", bufs=2)
                    nc.vector.tensor_copy(kt_stage[:], pk[:])
                    nc.sync.dma_start(kt_send[ch].ap()[ei * P:(ei + 1) * P, :],
                                      kt_stage[:])
                nc.gpsimd.collective_compute(
                    "AllGather", mybir.AluOpType.bypass,
                    replica_groups=REPLICA_GROUPS,
                    ins=[kt_send[ch].ap().opt()],
                    outs=[kt_allc[ch].ap().opt()],
                )

            # ---- v projection -> DRAM -> AllGather ----
            spc = ST // V_CHUNKS
            for ch in range(V_CHUNKS):
                for si in range(spc):
                    s = ch * spc + si
                    pv = ps1.tile([P, D], F32, name=f"pv{s}", tag="proj",
                                  bufs=2)
                    for d in range(DT):
                        for c in range(2):
                            nc.tensor.matmul(
                                pv[:, c * 512:(c + 1) * 512],
                                xT[d][:, s * P:(s + 1) * P],
                                wv_sb[d][:, c * 512:(c + 1) * 512],
                                start=(d == 0), stop=(d == DT - 1))
                    v_stage = sb.tile([P, D], BF16, name=f"vs{s}", tag="stage", bufs=2)
                    nc.vector.tensor_copy(v_stage[:], pv[:])
                    nc.sync.dma_start(v_send[ch].ap()[si * P:(si + 1) * P, :],
                                      v_stage[:])
                nc.gpsimd.collective_compute(
                    "AllGather", mybir.AluOpType.bypass,
                    replica_groups=REPLICA_GROUPS,
                    ins=[v_send[ch].ap().opt()],
                    outs=[v_allc[ch].ap().opt()],
                )

            # ---- q^T projection (wq was loaded at t0; no stalls) ----
            for e in range(DT):
                pq = ps1.tile([P, S_OWN], F32, name=f"pq{e}", tag="proj",
                              bufs=2)
                for d in range(DT):
                    for c in range(2):
                        nc.tensor.matmul(
                            pq[:, c * 512:(c + 1) * 512],
                            wq_sb[d][:, e * P:(e + 1) * P],
                            xT[d][:, c * 512:(c + 1) * 512],
                            start=(d == 0), stop=(d == DT - 1))
                nc.vector.tensor_copy(qT[e][:], pq[:])

        # ---- bring gathered k^T / v into SBUF ----
        # kTg ch0 -> wk slots (free at kT-proj end, data ready first);
        # kTg ch1 -> wq slots (free at q-proj end). Alloc order matters:
        # the wqwk tag was filled wq(0-7) then wk(8-15), so allocate 8
        # placeholders cycling back to wq slots last.
        kT_sb_flat = {}
        epc = DT // KT_CHUNKS
        for ch in range(KT_CHUNKS):
            for ei in range(epc):
                e = ch * epc + ei
                for r in range(2):
                    t = sb.tile([P, S_OWN], F32R, name=f"kTg{r}_{e}",
                                tag="wqwk", bufs=16)
                    kT_sb_flat[(r, e)] = t
                    eng = nc.sync if r == 0 else nc.scalar
                    eng.dma_start(
                        t[:], kt_allc[ch].ap()[r, ei * P:(ei + 1) * P, :])
        kT_sb = [[kT_sb_flat[(r, e)] for e in range(DT)] for r in range(2)]

        # vg: r0 into xT slots (free at q-proj end), r1 into wv slots
        v_sb = []
        for r in range(2):
            row = []
            for s in range(ST):
                t = sb.tile([P, D], BF16, name=f"vg{r}_{s}",
                            tag=("xTvg" if r == 0 else "wvvg"), bufs=8)
                row.append(t)
            v_sb.append(row)
        spc = ST // V_CHUNKS
        for ch in range(V_CHUNKS):
            for si in range(spc):
                s = ch * spc + si
                for r in range(2):
                    nc.sync.dma_start(
                        v_sb[r][s][:],
                        v_allc[ch].ap()[r, si * P:(si + 1) * P, :])

        # ---- attention, software-pipelined over 128-query tiles ----
        with tc.tile_pool(name="ps2", bufs=1, space="PSUM") as ps2:
            state = {}

            def emit_scores(sq):
                S_ps = ps2.tile([P, S_FULL], F32, name=f"S{sq}", tag="S",
                                bufs=1)
                for e in range(DT):
                    for r in range(2):
                        for c in range(2):
                            col = r * S_OWN + c * 512
                            nc.tensor.matmul(
                                S_ps[:, col:col + 512],
                                qT[e][:, sq * P:(sq + 1) * P],
                                kT_sb[r][e][:, c * 512:(c + 1) * 512],
                                start=(e == 0), stop=(e == DT - 1))
                state[sq] = S_ps

            def emit_softmax(sq):
                S_ps = state.pop(sq)
                m = sb.tile([P, 1], F32, name=f"m{sq}", tag="m", bufs=2)
                nc.vector.reduce_max(m[:], S_ps[:], axis=mybir.AxisListType.X)
                negm = sb.tile([P, 1], F32, name=f"negm{sq}", tag="negm",
                               bufs=2)
                nc.scalar.mul(negm[:], m[:], -1.0 / 32.0)
                attn = sb.tile([P, S_FULL], BF16, name=f"attn{sq}", tag="xa",
                               bufs=3)
                lsum = sb.tile([P, 1], F32, name=f"lsum{sq}", tag="lsum",
                               bufs=2)
                nc.scalar.activation(
                    attn[:], S_ps[:], mybir.ActivationFunctionType.Exp,
                    bias=negm[:, 0:1], scale=1.0 / 32.0, accum_out=lsum[:])
                rl = sb.tile([P, 1], F32, name=f"rl{sq}", tag="rl", bufs=2)
                nc.vector.reciprocal(rl[:], lsum[:])
                state[(sq, "sm")] = (attn, rl)

            def emit_transp(sq):
                attn, rl = state[(sq, "sm")]
                attnT = sb.tile([P, S_FULL], BF16, name=f"attnT{sq}",
                                tag="attnT", bufs=2)
                for t in range(NT):
                    pat = ps2.tile([P, P], BF16, name=f"pat{sq}_{t}",
                                   tag="pat", bufs=2)
                    nc.tensor.transpose(
                        pat[:], attn[:, t * P:(t + 1) * P], identb[:])
                    nc.vector.tensor_copy(attnT[:, t * P:(t + 1) * P], pat[:])
                state[(sq, "T")] = attnT

            def emit_avmm(sq):
                attn, rl = state.pop((sq, "sm"))
                attnT = state.pop((sq, "T"))
                O_ps = ps2.tile([P, D], F32, name=f"O{sq}", tag="O", bufs=1)
                for t in range(NT):
                    r, s = divmod(t, ST)
                    for c in range(2):
                        nc.tensor.matmul(
                            O_ps[:, c * 512:(c + 1) * 512],
                            attnT[:, t * P:(t + 1) * P],
                            v_sb[r][s][:, c * 512:(c + 1) * 512],
                            start=(t == 0), stop=(t == NT - 1))
                o_stage = sb.tile([P, D], F32, name=f"ost{sq}", tag="stage", bufs=2)
                nc.vector.tensor_scalar_mul(o_stage[:], O_ps[:], rl[:, 0:1])
                nc.sync.dma_start(out_d.ap()[sq * P:(sq + 1) * P, :],
                                  o_stage[:])

            # PE stream: T(i-1) | scores(i) | AV(i-1); the pat->attnT
            # copies run on DVE while scores(i) occupies the PE.
            emit_scores(0)
            emit_softmax(0)
            for sq in range(1, ST):
                emit_transp(sq - 1)
                emit_scores(sq)
                emit_avmm(sq - 1)
                emit_softmax(sq)
            emit_transp(ST - 1)
            emit_avmm(ST - 1)


_NC_CACHE = {}


def _get_nc():
    if "nc" not in _NC_CACHE:
        _NC_CACHE["nc"] = build_kernel()
    return _NC_CACHE["nc"]


def kernel(x, Wq, Wk, Wv, **_ignored):
    x = np.ascontiguousarray(np.asarray(x, dtype=np.float32))
    Wq = np.ascontiguousarray(np.asarray(Wq, dtype=np.float32))
    Wk = np.ascontiguousarray(np.asarray(Wk, dtype=np.float32))
    Wv = np.ascontiguousarray(np.asarray(Wv, dtype=np.float32))
    nc = _get_nc()
    in_maps = []
    for c in range(NCORES):
        b, h = divmod(c, 2)
        in_maps.append({
            "x": x[b, h * S_OWN:(h + 1) * S_OWN, :],
            "Wq": Wq, "Wk": Wk, "Wv": Wv,
        })
    res = run_bass_kernel_spmd(nc, in_maps, core_ids=list(range(NCORES)))
    out = np.empty((B, S_FULL, D), dtype=np.float32)
    for c in range(NCORES):
        b, h = divmod(c, 2)
        out[b, h * S_OWN:(h + 1) * S_OWN, :] = res.results[c]["out"]
    return out
